# revision 19
# baseline (speedup 1.0000x reference)
"""Trainium2 Bass kernel for nn_Attention_48095043781121 (v2).

Math (reference):
    q,k,v = x@Wq, x@Wk, x@Wv          (per head h: columns [64h, 64h+64))
    A     = softmax_j(q.k^T / 8)
    P[b,h,i,j] = softmax_j(-ph[b,j,h]) = w[b,h,j]   (independent of i)
    attn  = ((1-g)A + gP) / rowsum               rowsum == 1 exactly
    out   = attn @ v ;  y = concat_heads(out) @ Wo + bo

Per (b,h):  y-contribution = (1-g_h)/r * (E @ v_h)  +  g_h * (w @ v_h)
with E = exp(S/8), r[i] = sum_j E[i,j].

v2 design notes (all per core; 8 cores = 4 batches x 2 head-groups):
  * Every persistent matmul is a large-N fp16 matmul so the HAM clock
    gate ramps to 8/8 early and STAYS there (the v1 kernel's tiny
    attention matmuls dropped it to 4/8 for the last 34us).
  * E@v is computed transposed: stationary = v-tile [128j, 128cols],
    moving = E^T tile [128j, 512i] -> psum [(dh|r), 512i].  The v_aug
    column layout [v(64) | 1.0 | 0*31] per head makes head 2m land its
    output on psum partitions 0-64 and head 2m+1 (slice shifted -64)
    on partitions 64-127 (r at 32, 32-aligned for the DVE read), so a head PAIR assembles into one [128, S]
    feature-major oT tile with NO transposes, ready for the
    out-projection.  Row 64 (even) / 32 (odd) is r_i for free.
  * 1/r rows: DVE RECIPROCAL_APPROX_FAST (1 inst, ~51 ULP) + a
    (1-g_h) tensor_scalar fold; broadcast to 128 partitions with two
    K=1 matmuls into one psum bank (M=128 zero|ones trick).
  * gwv = g_h*(w@v_h) is constant over i, so it routes through the
    BIAS: gw_feat columns (2 tiny transposes) -> y_gwv[1,512] = gwv@Wo
    + bo in one accum chain, broadcast once, added during the psum->
    sbuf copy of the out-projection.  No per-head gwv work at all.
  * exp stays on ACT (the only exp engine): 8 ops of [128, 2*512]
    (two psum banks per op) to amortize the ~280ns/op overhead.
  * fp16 inputs (x, Wq/k/v, Wo) and fp16 y output halve DMA volume.
  * identity for transposes comes in via DMA (np.eye) -- v1 burned
    ~0.3us of GpSimd + barrier time building it on-chip.
"""

import numpy as np
from contextlib import ExitStack

B, S, DIM, H, DH = 4, 512, 512, 8, 64
POS_DIM, PD8 = 3, 64
NCORES = 8
HGH = 4           # heads per head-group (per core)
HGF = HGH * DH    # 256
KT = DIM // 128   # 4
MT = HGF // 128   # 2
ST = S // 128     # 4
DHP = 96          # [v(64) | 1.0 | 0*31]: odd-head r lands 32-aligned
NWARM = 8

_CACHE = {}
DEBUG = False


def _build_program():
    import concourse.bass as bass
    import concourse.mybir as mybir
    import concourse.tile as tile
    from concourse import bacc

    F32 = mybir.dt.float32
    F32R = mybir.dt.float32r
    F16 = mybir.dt.float16
    AF = mybir.ActivationFunctionType
    ALU = mybir.AluOpType

    nc = bacc.Bacc(trn_type="TRN2", target_bir_lowering=False, debug=False)

    xT_d = nc.dram_tensor("xT", [128, KT * S], F16, kind="ExternalInput")
    wq_d = nc.dram_tensor("Wq", [128, KT * HGF], F16, kind="ExternalInput")
    wk_d = nc.dram_tensor("Wk", [128, KT * HGF], F16, kind="ExternalInput")
    wv_d = nc.dram_tensor("Wv", [128, KT * HGF], F16, kind="ExternalInput")
    wo_d = nc.dram_tensor("Wo", [128, MT * DIM], F16, kind="ExternalInput")
    bo_d = nc.dram_tensor("bo", [1, DIM], F16, kind="ExternalInput")
    eye_d = nc.dram_tensor("eye", [128, 128], F16, kind="ExternalInput")
    posT_d = nc.dram_tensor("posT", [POS_DIM, S], F32R, kind="ExternalInput")
    wp1_d = nc.dram_tensor("Wp1", [POS_DIM, 4], F32R, kind="ExternalInput")
    bp1_d = nc.dram_tensor("bp1", [4], F32, kind="ExternalInput")
    wp2_d = nc.dram_tensor("Wp2", [POS_DIM, PD8], F32R, kind="ExternalInput")
    bp2_d = nc.dram_tensor("bp2", [PD8], F32, kind="ExternalInput")
    wh_d = nc.dram_tensor("Wh", [PD8, HGH], F32R, kind="ExternalInput")
    gr_d = nc.dram_tensor("gate_r", [1, HGH], F32, kind="ExternalInput")
    gc_d = nc.dram_tensor("gate_c", [HGH], F32, kind="ExternalInput")
    y_d = nc.dram_tensor("y", [S, DIM], F16, kind="ExternalOutput")
    if DEBUG:
        dbg_kT = nc.dram_tensor("dbg_kT", [128, MT * S], F16, kind="ExternalOutput")
        dbg_qT = nc.dram_tensor("dbg_qT", [128, MT * S], F16, kind="ExternalOutput")
        dbg_va = nc.dram_tensor("dbg_va", [128, ST * HGH * DHP], F16, kind="ExternalOutput")
        dbg_e0 = nc.dram_tensor("dbg_e0", [128, ST * S], F16, kind="ExternalOutput")
        dbg_rsc = nc.dram_tensor("dbg_rsc", [HGH, S], F16, kind="ExternalOutput")
        dbg_rb = nc.dram_tensor("dbg_rb", [128, 2 * S], F16, kind="ExternalOutput")
        dbg_oT = nc.dram_tensor("dbg_oT", [128, MT * S], F16, kind="ExternalOutput")
        dbg_w = nc.dram_tensor("dbg_w", [HGH, S], F16, kind="ExternalOutput")
        dbg_bo = nc.dram_tensor("dbg_bo", [1, DIM], F16, kind="ExternalOutput")
        dbg_rraw = nc.dram_tensor("dbg_rraw", [HGH, S], F32, kind="ExternalOutput")
        dbg_rr = nc.dram_tensor("dbg_rr", [HGH, S], F32, kind="ExternalOutput")
        dbg_wj = nc.dram_tensor("dbg_wj", [128, ST * HGH], F16, kind="ExternalOutput")
        dbg_gsb = nc.dram_tensor("dbg_gsb", [HGH, HGF], F16, kind="ExternalOutput")
        dbg_gwf = nc.dram_tensor("dbg_gwf", [128, MT], F16, kind="ExternalOutput")

    with tile.TileContext(nc) as tc, ExitStack() as ctx:
        sing = ctx.enter_context(tc.tile_pool(name="sing", bufs=1))
        epool = ctx.enter_context(tc.tile_pool(name="epool", bufs=3))
        ypool = ctx.enter_context(tc.tile_pool(name="ypool", bufs=2))
        # PSUM: 4 + 3 + 1 = 8 banks
        ps_sc = ctx.enter_context(tc.tile_pool(name="ps_sc", bufs=1, space="PSUM"))
        ps_ea = ctx.enter_context(tc.tile_pool(name="ps_ea", bufs=4, space="PSUM"))
        ps_sm = ctx.enter_context(tc.tile_pool(name="ps_sm", bufs=1, space="PSUM"))

        # ---- constants (DVE memsets; cheap, before its DMA issues) ----
        warm_w = sing.tile([128, 128], F16)
        nc.vector.memset(warm_w, 0.25)
        warm_s = sing.tile([128, 512], F16)
        nc.vector.memset(warm_s, 0.5)
        ones1h = sing.tile([1, 128], F16)
        nc.vector.memset(ones1h, 1.0)
        onesB = sing.tile([1, 128], F16)
        nc.vector.memset(onesB, 0.0)
        nc.vector.memset(onesB[:, 64:128], 1.0)
        v_aug = sing.tile([128, ST, HGH, DHP], F16)
        nc.vector.memset(v_aug[:, :, :, DH:DHP], 0.0)
        nc.vector.memset(v_aug[:, :, :, DH : DH + 1], 1.0)
        v_flat = v_aug.rearrange("p t h c -> p t (h c)")

        # ---- input DMAs (issue engines: sync=bigs, gpsimd=wq/wv,
        #      vector=pos smalls; ACT issues nothing -- it's the exp
        #      bottleneck) ----
        xT = sing.tile([128, KT, S], F16)
        wk = sing.tile([128, KT, HGF], F16)
        wq = sing.tile([128, KT, HGF], F16)
        wv = sing.tile([128, KT, HGF], F16)
        wo = sing.tile([128, MT, DIM], F16)
        bo_row = sing.tile([1, DIM], F16)
        eye = sing.tile([128, 128], F16)
        nc.sync.dma_start(out=xT[:, 0:2, :], in_=xT_d.ap()[:, 0 : 2 * S])
        nc.sync.dma_start(out=wk, in_=wk_d.ap())
        nc.sync.dma_start(out=xT[:, 2:KT, :], in_=xT_d.ap()[:, 2 * S : KT * S])
        nc.sync.dma_start(out=wq, in_=wq_d.ap())
        nc.sync.dma_start(out=wv, in_=wv_d.ap())
        nc.sync.dma_start(out=eye, in_=eye_d.ap())
        nc.sync.dma_start(out=wo, in_=wo_d.ap())
        nc.sync.dma_start(out=bo_row, in_=bo_d.ap())
        posT = sing.tile([POS_DIM, S], F32R)
        nc.scalar.dma_start(out=posT, in_=posT_d.ap())
        wp1 = sing.tile([POS_DIM, 4], F32R)
        nc.scalar.dma_start(out=wp1, in_=wp1_d.ap())
        bp1 = sing.tile([4, 1], F32)
        nc.scalar.dma_start(out=bp1, in_=bp1_d.ap()[:, None])
        gate_r = sing.tile([1, HGH], F32)
        nc.scalar.dma_start(out=gate_r, in_=gr_d.ap())
        gate_c = sing.tile([HGH, 1], F32)
        nc.scalar.dma_start(out=gate_c, in_=gc_d.ap()[:, None])
        wp2 = sing.tile([POS_DIM, PD8], F32R)
        nc.scalar.dma_start(out=wp2, in_=wp2_d.ap())
        bp2 = sing.tile([PD8, 1], F32)
        nc.scalar.dma_start(out=bp2, in_=bp2_d.ap()[:, None])
        wh = sing.tile([PD8, HGH], F32R)
        nc.scalar.dma_start(out=wh, in_=wh_d.ap())

        # ---- warmup: keeps PE duty high from ~0.5us so the HAM gate
        # flips to 8/8 during the DMA head; pos-path matmuls interleave
        # so the ACT ping-pong hides inside it ----
        def warm(n):
            for _ in range(n):
                wps = ps_ea.tile([128, 512], F32, tag="ea")
                nc.tensor.matmul(wps, warm_w, warm_s, start=True, stop=True)

        with nc.named_scope("pos_warm"):
            # one unbroken burst: the HAM boost (fixed ~17us of 8/8 clock)
            # is granted after ~5us of CONTINUOUS PE duty; any stall
            # resets the counter, so nothing DMA- or ACT-dependent may
            # interleave here.
            warm(NWARM)
            p1ps = ps_sm.tile([4, S], F32, tag="sm")
            nc.tensor.matmul(p1ps, wp1, posT, start=True, stop=True)

            p1 = sing.tile([4, S], F32R)
            nc.scalar.activation(p1, p1ps, AF.Relu, bias=bp1)

        # pos-MLP layers 2/3 + gate math, emitted interleaved with the
        # projections so the ACT round-trips hide behind real PE work
        omg_r = sing.tile([1, HGH], F32)  # 1-g_h = 1/(1+e^g)
        g_c = sing.tile([HGH, 1], F32)
        w_sb = sing.tile([HGH, S], F16)
        wj = sing.tile([128, ST, HGH], F16)
        _pos = {}

        def pos_l2():
            p2ps = ps_sm.tile([PD8, S], F32, tag="sm")
            nc.tensor.matmul(p2ps, wp2, p1[0:POS_DIM, :], start=True, stop=True)
            p2 = sing.tile([PD8, S], F32R)
            nc.scalar.activation(p2, p2ps, AF.Identity, bias=bp2)
            _pos["p2"] = p2

        def pos_l3():
            phps = ps_sm.tile([HGH, S], F32, tag="sm")
            nc.tensor.matmul(phps, wh, _pos["p2"], start=True, stop=True)
            # gate math: g = sigmoid(gate) via exp (same ACT table)
            eg_r = sing.tile([1, HGH], F32)
            nc.scalar.activation(eg_r, gate_r, AF.Exp)
            eg_c = sing.tile([HGH, 1], F32)
            nc.scalar.activation(eg_c, gate_c, AF.Exp)
            wexp = sing.tile([HGH, S], F32)
            wsum = sing.tile([HGH, 1], F32)
            nc.scalar.activation(wexp, phps, AF.Exp, scale=-1.0, accum_out=wsum)
            s_r = sing.tile([1, HGH], F32)
            nc.vector.tensor_scalar_add(s_r, eg_r, 1.0)
            nc.vector.reciprocal(omg_r, s_r)
            s_c = sing.tile([HGH, 1], F32)
            nc.vector.tensor_scalar_add(s_c, eg_c, 1.0)
            is_c = sing.tile([HGH, 1], F32)
            nc.vector.reciprocal(is_c, s_c)
            nc.vector.tensor_mul(g_c, eg_c, is_c)
            winv = sing.tile([HGH, 1], F32)
            nc.vector.reciprocal(winv, wsum)
            gwin = sing.tile([HGH, 1], F32)
            nc.vector.tensor_mul(gwin, winv, g_c)
            nc.vector.tensor_scalar_mul(w_sb, wexp, gwin)

        def wj_transposes():
            # w transposed: wj[j, h] per j-tile (gwv stationary)
            for jt in range(ST):
                wt = ps_sm.tile([128, HGH], F16, tag="sm")
                nc.tensor.transpose(
                    wt, w_sb[:, 128 * jt : 128 * (jt + 1)], eye[0:HGH, 0:HGH]
                )
                nc.vector.tensor_copy(wj[:, jt, :], wt)

        # ---- projections ----
        kT_sb = sing.tile([128, MT, S], F16)
        qT_sb = sing.tile([128, MT, S], F16)

        def proj_one(dst, w, m, on_act):
            ps = ps_ea.tile([128, S], F32, tag="ea")
            for kk in range(KT):
                nc.tensor.matmul(
                    ps,
                    w[:, kk, 128 * m : 128 * (m + 1)],
                    xT[:, kk, :],
                    start=(kk == 0),
                    stop=(kk == KT - 1),
                )
            if on_act:
                nc.scalar.activation(dst[:, m, :], ps, AF.Identity)
            else:
                nc.vector.tensor_copy(dst[:, m, :], ps)

        def proj_v():
            for tt in range(ST):
                ps = ps_ea.tile([128, HGF], F32, tag="ea")
                for kk in range(KT):
                    nc.tensor.matmul(
                        ps,
                        xT[:, kk, 128 * tt : 128 * (tt + 1)],
                        wv[:, kk, :],
                        start=(kk == 0),
                        stop=(kk == KT - 1),
                    )
                nc.vector.tensor_copy(
                    v_aug[:, tt, :, 0:DH], ps.rearrange("p (h c) -> p h c", c=DH)
                )

        # ---- attention pieces ----
        def scores(h):
            m, sub = h // 2, h % 2
            e_h = epool.tile([128, ST, S], F16, tag="e")
            for half in range(2):
                sps = ps_sc.tile([128, 2, S], F32, tag="sc")
                for j2 in range(2):
                    jt = 2 * half + j2
                    nc.tensor.matmul(
                        sps[:, j2, :],
                        kT_sb[64 * sub : 64 * sub + 64, m, 128 * jt : 128 * (jt + 1)],
                        qT_sb[64 * sub : 64 * sub + 64, m, :],
                        start=True,
                        stop=True,
                    )
                nc.scalar.activation(
                    e_h[:, 2 * half : 2 * half + 2, :], sps, AF.Exp, scale=0.125
                )
            return e_h

        def eav(h, e_h):
            # stationary slice: even h -> [66h, 66h+128): v at psum 0-63,
            # r at 64; odd h -> [66h-64, 66h+64): r at 62, v at 64-127.
            ps = ps_ea.tile([128, S], F32, tag="ea")
            off = DHP * h - (64 if h % 2 else 0)
            for jt in range(ST):
                nc.tensor.matmul(
                    ps,
                    v_flat[:, jt, off : off + 128],
                    e_h[:, jt, :],
                    start=(jt == 0),
                    stop=(jt == ST - 1),
                )
            return ps

        def rsc_row(h, ps, copy_on_act=False):
            rrow = 64 if h % 2 == 0 else 32
            rcp = sing.tile([1, S], F32, tag=f"rcp{h}")
            if copy_on_act:
                nc.scalar.activation(rcp, ps[rrow : rrow + 1, :], AF.Identity)
            else:
                nc.vector.tensor_copy(rcp, ps[rrow : rrow + 1, :])
            rr = sing.tile([1, S], F32, tag=f"rr{h}")
            nc.vector.reciprocal_approx_fast(rr, rcp)
            rsc = sing.tile([1, S], F16, tag=f"rsc{h}")
            nc.vector.tensor_scalar_mul(rsc, rr, omg_r[:, h : h + 1])
            if DEBUG:
                nc.sync.dma_start(out=dbg_rraw.ap()[h : h + 1, :], in_=rcp)
                nc.sync.dma_start(out=dbg_rr.ap()[h : h + 1, :], in_=rr)
            return rsc

        def bcast_pair(pp, rsc_a, rsc_b):
            rb = ps_sm.tile([128, S], F32, tag="sm")
            nc.tensor.matmul(rb, onesB, rsc_b, start=True, stop=False)
            nc.tensor.matmul(
                rb[0:64, :],
                ones1h[0:1, 0:64],
                rsc_a,
                start=False,
                stop=True,
                skip_group_check=True,
            )
            rb_sb = sing.tile([128, S], F16, tag=f"rb{pp}")
            if pp == 1:
                nc.scalar.activation(rb_sb, rb, AF.Identity)
            else:
                nc.vector.tensor_copy(rb_sb, rb)
            return rb_sb

        oT = sing.tile([128, MT, S], F16)

        def finish_pair(m, ps_a, ps_b, rb):
            nc.vector.tensor_mul(oT[0:64, m, :], ps_a[0:64, :], rb[0:64, :])
            nc.vector.tensor_mul(oT[64:128, m, :], ps_b[64:128, :], rb[64:128, :])

        # ---- gwv -> bias fold ----
        bo_eff = sing.tile([1, DIM], F16)
        g_sb_dbg, gw_feat_dbg = [], []

        def gwv_path():
            gps = ps_sm.tile([HGH, HGF], F32, tag="sm")
            for jt in range(ST):
                nc.tensor.matmul(
                    gps,
                    wj[:, jt, :],
                    v_aug[:, jt, :, 0:DH],
                    start=(jt == 0),
                    stop=(jt == ST - 1),
                )
            g_sb = sing.tile([HGH, HGF], F16)
            nc.vector.tensor_copy(g_sb, gps)
            g_sb_dbg.append(g_sb)
            gw_feat = sing.tile([128, MT], F16)
            gw_feat_dbg.append(gw_feat)
            for fm in range(MT):
                tp = ps_sm.tile([128, HGH], F16, tag="sm")
                nc.tensor.transpose(
                    tp, g_sb[:, 128 * fm : 128 * (fm + 1)], eye[0:HGH, 0:HGH]
                )
                nc.vector.tensor_copy(
                    gw_feat[0:64, fm : fm + 1], tp[0:64, 2 * fm : 2 * fm + 1]
                )
                nc.vector.tensor_copy(
                    gw_feat[64:128, fm : fm + 1], tp[64:128, 2 * fm + 1 : 2 * fm + 2]
                )
            yg = ps_sm.tile([1, DIM], F32, tag="sm")
            nc.tensor.matmul(yg, gw_feat[:, 0:1], wo[:, 0, :], start=True, stop=False)
            nc.tensor.matmul(yg, gw_feat[:, 1:2], wo[:, 1, :], start=False, stop=False)
            nc.tensor.matmul(yg, ones1h[0:1, 0:1], bo_row, start=False, stop=True)
            nc.vector.tensor_copy(bo_eff, yg)

        # ---- out-projection: bias via K=1 matmul, y DMA straight from
        # PSUM (fp32; host casts/sums) ----
        def outproj(it):
            yps = ps_ea.tile([128, DIM], F32, tag="ea")
            for fm in range(MT):
                nc.tensor.matmul(
                    yps,
                    oT[:, fm, 128 * it : 128 * (it + 1)],
                    wo[:, fm, :],
                    start=(fm == 0),
                    stop=False,
                )
            nc.tensor.matmul(yps, ones1h, bo_eff, start=False, stop=True)
            ysb = ypool.tile([128, DIM], F16, tag="y")
            if it % 2 == 0:
                nc.scalar.activation(ysb, yps, AF.Identity)
            else:
                nc.vector.tensor_copy(ysb, yps)
            nc.sync.dma_start(out=y_d.ap()[128 * it : 128 * (it + 1), :], in_=ysb)

        # ---- main sequence (PE emission order == PE queue order).
        # All scores go as early as possible: the ACT exp stream is the
        # pipeline pacer and must never starve.  eav/gwv/bcast fill the
        # PE while exps run; the pair-1 tail row ops ride on ACT (idle
        # after the last exp).
        with nc.named_scope("proj"):
            proj_one(kT_sb, wk, 0, True)
            pos_l2()
            proj_one(qT_sb, wq, 0, False)
            pos_l3()
        with nc.named_scope("attn"):
            e0 = scores(0)
            proj_one(kT_sb, wk, 1, False)
            proj_one(qT_sb, wq, 1, False)
            e1 = scores(1)
            with nc.named_scope("projv"):
                proj_v()
            e2 = scores(2)
            wj_transposes()
            ps_a0 = eav(0, e0)
            rsc0 = rsc_row(0, ps_a0)
            with nc.named_scope("gwv"):
                gwv_path()
            ps_b0 = eav(1, e1)
            rsc1 = rsc_row(1, ps_b0)
            rb0 = bcast_pair(0, rsc0, rsc1)
            e3 = scores(3)
            finish_pair(0, ps_a0, ps_b0, rb0)
            ps_a1 = eav(2, e2)
            rsc2 = rsc_row(2, ps_a1)
            ps_b1 = eav(3, e3)
            rsc3 = rsc_row(3, ps_b1, copy_on_act=True)
            rb1 = bcast_pair(1, rsc2, rsc3)
            finish_pair(1, ps_a1, ps_b1, rb1)
        with nc.named_scope("outproj"):
            for it in range(ST):
                outproj(it)
    nc.compile()
    return nc


def _get_program():
    if "nc" not in _CACHE:
        _CACHE["nc"] = _build_program()
    return _CACHE["nc"]


def _ktile(a, dtype=np.float16):
    # [K*128, n] -> [128, K*n] (per-partition-contiguous k-tile layout)
    k = a.shape[0] // 128
    return np.ascontiguousarray(
        a.reshape(k, 128, a.shape[1]).transpose(1, 0, 2).reshape(128, -1).astype(dtype)
    )


def _make_in_maps(inputs):
    f = lambda a: np.ascontiguousarray(np.asarray(a), dtype=np.float32)
    x = f(inputs["x"])
    pos = f(inputs["pos"])
    Wq, Wk, Wv, Wo = f(inputs["Wq"]), f(inputs["Wk"]), f(inputs["Wv"]), f(inputs["Wo"])
    bo = f(inputs["bo"])
    Wp1, bp1 = f(inputs["Wp1"]), f(inputs["bp1"])
    Wp2, bp2 = f(inputs["Wp2"]), f(inputs["bp2"])
    Wh, gate = f(inputs["Wh"]), f(inputs["gate"])
    # pad the tiny pos-MLP first layer to 4 outputs (fp32r even-size rule)
    Wp1 = np.concatenate([Wp1, np.zeros((POS_DIM, 1), np.float32)], axis=1)
    bp1 = np.concatenate([bp1, np.zeros(1, np.float32)])
    eye = np.eye(128, dtype=np.float16)

    in_maps = []
    for c in range(NCORES):
        b, hg = c // 2, c % 2
        cs = slice(HGF * hg, HGF * (hg + 1))
        in_maps.append(
            {
                "xT": _ktile(x[b].T),
                "Wq": _ktile(Wq[:, cs]),
                "Wk": _ktile(Wk[:, cs]),
                "Wv": _ktile(Wv[:, cs]),
                "Wo": _ktile(Wo[cs, :]),
                "bo": (bo if hg == 0 else np.zeros_like(bo)).astype(np.float16)[None, :],
                "eye": eye,
                "posT": np.ascontiguousarray(pos[b].T),
                "Wp1": Wp1,
                "bp1": bp1,
                "Wp2": Wp2,
                "bp2": bp2,
                "Wh": np.ascontiguousarray(Wh[:, HGH * hg : HGH * (hg + 1)]),
                "gate_r": np.ascontiguousarray(
                    gate[HGH * hg : HGH * (hg + 1)][None, :]
                ),
                "gate_c": np.ascontiguousarray(gate[HGH * hg : HGH * (hg + 1)]),
            }
        )
    return in_maps


def run(inputs, trace=False):
    """Run on 8 NeuronCores; returns (out [B,S,DIM] fp32, BassKernelResults)."""
    from concourse.bass_utils import run_bass_kernel_spmd

    nc = _get_program()
    in_maps = _make_in_maps(inputs)
    res = run_bass_kernel_spmd(
        nc, in_maps, core_ids=list(range(NCORES)), trace=trace
    )
    out = np.empty((B, S, DIM), np.float32)
    for b in range(B):
        out[b] = res.results[2 * b]["y"].astype(np.float32) + res.results[
            2 * b + 1
        ]["y"].astype(np.float32)
    return out, res


def kernel(**inputs):
    out, _ = run(inputs, trace=False)
    return out


# revision 20
# speedup vs baseline: 1.1703x; 1.1703x over previous
"""Trainium2 Bass kernel for nn_Attention_48095043781121 (v2).

Math (reference):
    q,k,v = x@Wq, x@Wk, x@Wv          (per head h: columns [64h, 64h+64))
    A     = softmax_j(q.k^T / 8)
    P[b,h,i,j] = softmax_j(-ph[b,j,h]) = w[b,h,j]   (independent of i)
    attn  = ((1-g)A + gP) / rowsum               rowsum == 1 exactly
    out   = attn @ v ;  y = concat_heads(out) @ Wo + bo

Per (b,h):  y-contribution = (1-g_h)/r * (E @ v_h)  +  g_h * (w @ v_h)
with E = exp(S/8), r[i] = sum_j E[i,j].

v2 design notes (all per core; 8 cores = 4 batches x 2 head-groups):
  * Every persistent matmul is a large-N fp16 matmul so the HAM clock
    gate ramps to 8/8 early and STAYS there (the v1 kernel's tiny
    attention matmuls dropped it to 4/8 for the last 34us).
  * E@v is computed transposed: stationary = v-tile [128j, 128cols],
    moving = E^T tile [128j, 512i] -> psum [(dh|r), 512i].  The v_aug
    column layout [v(64) | 1.0 | 0*31] per head makes head 2m land its
    output on psum partitions 0-64 and head 2m+1 (slice shifted -64)
    on partitions 64-127 (r at 32, 32-aligned for the DVE read), so a head PAIR assembles into one [128, S]
    feature-major oT tile with NO transposes, ready for the
    out-projection.  Row 64 (even) / 32 (odd) is r_i for free.
  * 1/r rows: DVE RECIPROCAL_APPROX_FAST (1 inst, ~51 ULP) + a
    (1-g_h) tensor_scalar fold; broadcast to 128 partitions with two
    K=1 matmuls into one psum bank (M=128 zero|ones trick).
  * gwv = g_h*(w@v_h) is constant over i, so it routes through the
    BIAS: gw_feat columns (2 tiny transposes) -> y_gwv[1,512] = gwv@Wo
    + bo in one accum chain, broadcast once, added during the psum->
    sbuf copy of the out-projection.  No per-head gwv work at all.
  * exp stays on ACT (the only exp engine): 8 ops of [128, 2*512]
    (two psum banks per op) to amortize the ~280ns/op overhead.
  * fp16 inputs (x, Wq/k/v, Wo) and fp16 y output halve DMA volume.
  * identity for transposes comes in via DMA (np.eye) -- v1 burned
    ~0.3us of GpSimd + barrier time building it on-chip.
"""

import numpy as np
from contextlib import ExitStack

B, S, DIM, H, DH = 4, 512, 512, 8, 64
POS_DIM, PD8 = 3, 64
NCORES = 8
HGH = 4           # heads per head-group (per core)
HGF = HGH * DH    # 256
KT = DIM // 128   # 4
MT = HGF // 128   # 2
ST = S // 128     # 4
DHP = 96          # [v(64) | 1.0 | 0*31]: odd-head r lands 32-aligned
NWARM = 11

_CACHE = {}
DEBUG = False


def _build_program():
    import concourse.bass as bass
    import concourse.mybir as mybir
    import concourse.tile as tile
    from concourse import bacc

    F32 = mybir.dt.float32
    F32R = mybir.dt.float32r
    F16 = mybir.dt.float16
    AF = mybir.ActivationFunctionType
    ALU = mybir.AluOpType

    nc = bacc.Bacc(trn_type="TRN2", target_bir_lowering=False, debug=False)

    xT_d = nc.dram_tensor("xT", [128, KT * S], F16, kind="ExternalInput")
    wq_d = nc.dram_tensor("Wq", [128, KT * HGF], F16, kind="ExternalInput")
    wk_d = nc.dram_tensor("Wk", [128, KT * HGF], F16, kind="ExternalInput")
    wv_d = nc.dram_tensor("Wv", [128, KT * HGF], F16, kind="ExternalInput")
    wo_d = nc.dram_tensor("Wo", [128, MT * DIM], F16, kind="ExternalInput")
    bo_d = nc.dram_tensor("bo", [1, DIM], F16, kind="ExternalInput")
    eye_d = nc.dram_tensor("eye", [128, 128], F16, kind="ExternalInput")
    posT_d = nc.dram_tensor("posT", [POS_DIM, S], F32R, kind="ExternalInput")
    wp1_d = nc.dram_tensor("Wp1", [POS_DIM, 4], F32R, kind="ExternalInput")
    bp1_d = nc.dram_tensor("bp1", [4], F32, kind="ExternalInput")
    wp2_d = nc.dram_tensor("Wp2", [POS_DIM, PD8], F32R, kind="ExternalInput")
    bp2_d = nc.dram_tensor("bp2", [PD8], F32, kind="ExternalInput")
    wh_d = nc.dram_tensor("Wh", [PD8, HGH], F32R, kind="ExternalInput")
    gr_d = nc.dram_tensor("gate_r", [1, HGH], F32, kind="ExternalInput")
    gc_d = nc.dram_tensor("gate_c", [HGH], F32, kind="ExternalInput")
    y_d = nc.dram_tensor("y", [S, DIM], F16, kind="ExternalOutput")
    if DEBUG:
        dbg_kT = nc.dram_tensor("dbg_kT", [128, MT * S], F16, kind="ExternalOutput")
        dbg_qT = nc.dram_tensor("dbg_qT", [128, MT * S], F16, kind="ExternalOutput")
        dbg_va = nc.dram_tensor("dbg_va", [128, ST * HGH * DHP], F16, kind="ExternalOutput")
        dbg_e0 = nc.dram_tensor("dbg_e0", [128, ST * S], F16, kind="ExternalOutput")
        dbg_rsc = nc.dram_tensor("dbg_rsc", [HGH, S], F16, kind="ExternalOutput")
        dbg_rb = nc.dram_tensor("dbg_rb", [128, 2 * S], F16, kind="ExternalOutput")
        dbg_oT = nc.dram_tensor("dbg_oT", [128, MT * S], F16, kind="ExternalOutput")
        dbg_w = nc.dram_tensor("dbg_w", [HGH, S], F16, kind="ExternalOutput")
        dbg_bo = nc.dram_tensor("dbg_bo", [1, DIM], F16, kind="ExternalOutput")
        dbg_rraw = nc.dram_tensor("dbg_rraw", [HGH, S], F32, kind="ExternalOutput")
        dbg_rr = nc.dram_tensor("dbg_rr", [HGH, S], F32, kind="ExternalOutput")
        dbg_wj = nc.dram_tensor("dbg_wj", [128, ST * HGH], F16, kind="ExternalOutput")
        dbg_gsb = nc.dram_tensor("dbg_gsb", [HGH, HGF], F16, kind="ExternalOutput")
        dbg_gwf = nc.dram_tensor("dbg_gwf", [128, MT], F16, kind="ExternalOutput")

    with tile.TileContext(nc) as tc, ExitStack() as ctx:
        sing = ctx.enter_context(tc.tile_pool(name="sing", bufs=1))
        epool = ctx.enter_context(tc.tile_pool(name="epool", bufs=3))
        ypool = ctx.enter_context(tc.tile_pool(name="ypool", bufs=2))
        # PSUM: 4 + 3 + 1 = 8 banks
        ps_sc = ctx.enter_context(tc.tile_pool(name="ps_sc", bufs=2, space="PSUM"))
        ps_ea = ctx.enter_context(tc.tile_pool(name="ps_ea", bufs=3, space="PSUM"))
        ps_sm = ctx.enter_context(tc.tile_pool(name="ps_sm", bufs=1, space="PSUM"))

        # ---- constants (DVE memsets; cheap, before its DMA issues) ----
        warm_w = sing.tile([128, 128], F16)
        nc.vector.memset(warm_w, 0.25)
        warm_s = sing.tile([128, 512], F16)
        nc.vector.memset(warm_s, 0.5)
        ones1h = sing.tile([1, 128], F16)
        nc.vector.memset(ones1h, 1.0)
        onesB = sing.tile([1, 128], F16)
        nc.vector.memset(onesB, 0.0)
        nc.vector.memset(onesB[:, 64:128], 1.0)
        v_aug = sing.tile([128, ST, HGH, DHP], F16)
        nc.vector.memset(v_aug[:, :, :, DH:DHP], 0.0)
        nc.vector.memset(v_aug[:, :, :, DH : DH + 1], 1.0)
        v_flat = v_aug.rearrange("p t h c -> p t (h c)")

        # ---- input DMAs (issue engines: sync=bigs, gpsimd=wq/wv,
        #      vector=pos smalls; ACT issues nothing -- it's the exp
        #      bottleneck) ----
        xT = sing.tile([128, KT, S], F16)
        wk = sing.tile([128, KT, HGF], F16)
        wq = sing.tile([128, KT, HGF], F16)
        wv = sing.tile([128, KT, HGF], F16)
        wo = sing.tile([128, MT, DIM], F16)
        bo_row = sing.tile([1, DIM], F16)
        eye = sing.tile([128, 128], F16)
        nc.sync.dma_start(out=xT[:, 0:2, :], in_=xT_d.ap()[:, 0 : 2 * S])
        nc.sync.dma_start(out=wk, in_=wk_d.ap())
        nc.sync.dma_start(out=xT[:, 2:KT, :], in_=xT_d.ap()[:, 2 * S : KT * S])
        nc.sync.dma_start(out=wq, in_=wq_d.ap())
        nc.sync.dma_start(out=wv, in_=wv_d.ap())
        nc.sync.dma_start(out=eye, in_=eye_d.ap())
        nc.sync.dma_start(out=wo, in_=wo_d.ap())
        nc.sync.dma_start(out=bo_row, in_=bo_d.ap())
        posT = sing.tile([POS_DIM, S], F32R)
        nc.scalar.dma_start(out=posT, in_=posT_d.ap())
        wp1 = sing.tile([POS_DIM, 4], F32R)
        nc.scalar.dma_start(out=wp1, in_=wp1_d.ap())
        bp1 = sing.tile([4, 1], F32)
        nc.scalar.dma_start(out=bp1, in_=bp1_d.ap()[:, None])
        gate_r = sing.tile([1, HGH], F32)
        nc.scalar.dma_start(out=gate_r, in_=gr_d.ap())
        gate_c = sing.tile([HGH, 1], F32)
        nc.scalar.dma_start(out=gate_c, in_=gc_d.ap()[:, None])
        wp2 = sing.tile([POS_DIM, PD8], F32R)
        nc.scalar.dma_start(out=wp2, in_=wp2_d.ap())
        bp2 = sing.tile([PD8, 1], F32)
        nc.scalar.dma_start(out=bp2, in_=bp2_d.ap()[:, None])
        wh = sing.tile([PD8, HGH], F32R)
        nc.scalar.dma_start(out=wh, in_=wh_d.ap())

        # ---- warmup: keeps PE duty high from ~0.5us so the HAM gate
        # flips to 8/8 during the DMA head; pos-path matmuls interleave
        # so the ACT ping-pong hides inside it ----
        def warm(n):
            for _ in range(n):
                wps = ps_ea.tile([128, 512], F32, tag="ea")
                nc.tensor.matmul(wps, warm_w, warm_s, start=True, stop=True)

        with nc.named_scope("pos_warm"):
            # one unbroken burst: the HAM boost (fixed ~17us of 8/8 clock)
            # is granted after ~5us of CONTINUOUS PE duty; any stall
            # resets the counter, so nothing DMA- or ACT-dependent may
            # interleave here.
            warm(NWARM)
            p1ps = ps_sm.tile([4, S], F32, tag="sm")
            nc.tensor.matmul(p1ps, wp1, posT, start=True, stop=True)

            p1 = sing.tile([4, S], F32R)
            nc.scalar.activation(p1, p1ps, AF.Relu, bias=bp1)

        # pos-MLP layers 2/3 + gate math, emitted interleaved with the
        # projections so the ACT round-trips hide behind real PE work
        omg_r = sing.tile([1, HGH], F32)  # 1-g_h = 1/(1+e^g)
        g_c = sing.tile([HGH, 1], F32)
        w_sb = sing.tile([HGH, S], F16)
        wj = sing.tile([128, ST, HGH], F16)
        _pos = {}

        def pos_l2():
            p2ps = ps_sm.tile([PD8, S], F32, tag="sm")
            nc.tensor.matmul(p2ps, wp2, p1[0:POS_DIM, :], start=True, stop=True)
            p2 = sing.tile([PD8, S], F32R)
            nc.scalar.activation(p2, p2ps, AF.Identity, bias=bp2)
            _pos["p2"] = p2

        def pos_l3():
            phps = ps_sm.tile([HGH, S], F32, tag="sm")
            nc.tensor.matmul(phps, wh, _pos["p2"], start=True, stop=True)
            # gate math: g = sigmoid(gate) via exp (same ACT table)
            eg_r = sing.tile([1, HGH], F32)
            nc.scalar.activation(eg_r, gate_r, AF.Exp)
            eg_c = sing.tile([HGH, 1], F32)
            nc.scalar.activation(eg_c, gate_c, AF.Exp)
            wexp = sing.tile([HGH, S], F32)
            wsum = sing.tile([HGH, 1], F32)
            nc.scalar.activation(wexp, phps, AF.Exp, scale=-1.0, accum_out=wsum)
            s_r = sing.tile([1, HGH], F32)
            nc.vector.tensor_scalar_add(s_r, eg_r, 1.0)
            nc.vector.reciprocal(omg_r, s_r)
            s_c = sing.tile([HGH, 1], F32)
            nc.vector.tensor_scalar_add(s_c, eg_c, 1.0)
            is_c = sing.tile([HGH, 1], F32)
            nc.vector.reciprocal(is_c, s_c)
            nc.vector.tensor_mul(g_c, eg_c, is_c)
            winv = sing.tile([HGH, 1], F32)
            nc.vector.reciprocal(winv, wsum)
            gwin = sing.tile([HGH, 1], F32)
            nc.vector.tensor_mul(gwin, winv, g_c)
            nc.vector.tensor_scalar_mul(w_sb, wexp, gwin)

        def wj_transposes():
            # w transposed: wj[j, h] per j-tile (gwv stationary)
            for jt in range(ST):
                wt = ps_sm.tile([128, HGH], F16, tag="sm")
                nc.tensor.transpose(
                    wt, w_sb[:, 128 * jt : 128 * (jt + 1)], eye[0:HGH, 0:HGH]
                )
                nc.vector.tensor_copy(wj[:, jt, :], wt)

        # ---- projections ----
        kT_sb = sing.tile([128, MT, S], F16)
        qT_sb = sing.tile([128, MT, S], F16)

        def proj_one(dst, w, m, on_act):
            ps = ps_ea.tile([128, S], F32, tag="ea")
            for kk in range(KT):
                nc.tensor.matmul(
                    ps,
                    w[:, kk, 128 * m : 128 * (m + 1)],
                    xT[:, kk, :],
                    start=(kk == 0),
                    stop=(kk == KT - 1),
                )
            if on_act:
                nc.scalar.activation(dst[:, m, :], ps, AF.Identity)
            else:
                nc.vector.tensor_copy(dst[:, m, :], ps)

        def proj_v():
            for tt in range(ST):
                ps = ps_ea.tile([128, HGF], F32, tag="ea")
                for kk in range(KT):
                    nc.tensor.matmul(
                        ps,
                        xT[:, kk, 128 * tt : 128 * (tt + 1)],
                        wv[:, kk, :],
                        start=(kk == 0),
                        stop=(kk == KT - 1),
                    )
                nc.vector.tensor_copy(
                    v_aug[:, tt, :, 0:DH], ps.rearrange("p (h c) -> p h c", c=DH)
                )

        # ---- attention pieces ----
        def scores(h):
            m, sub = h // 2, h % 2
            e_h = epool.tile([128, ST, S], F16, tag="e")
            for half in range(2):
                sps = ps_sc.tile([128, 2, S], F32, tag="sc")
                for j2 in range(2):
                    jt = 2 * half + j2
                    nc.tensor.matmul(
                        sps[:, j2, :],
                        kT_sb[64 * sub : 64 * sub + 64, m, 128 * jt : 128 * (jt + 1)],
                        qT_sb[64 * sub : 64 * sub + 64, m, :],
                        start=True,
                        stop=True,
                    )
                nc.scalar.activation(
                    e_h[:, 2 * half : 2 * half + 2, :], sps, AF.Exp, scale=0.125
                )
            return e_h

        def eav(h, e_h):
            # stationary slice: even h -> [66h, 66h+128): v at psum 0-63,
            # r at 64; odd h -> [66h-64, 66h+64): r at 62, v at 64-127.
            ps = ps_ea.tile([128, S], F32, tag="ea")
            off = DHP * h - (64 if h % 2 else 0)
            for jt in range(ST):
                nc.tensor.matmul(
                    ps,
                    v_flat[:, jt, off : off + 128],
                    e_h[:, jt, :],
                    start=(jt == 0),
                    stop=(jt == ST - 1),
                )
            return ps

        def rsc_row(h, ps, copy_on_act=False):
            rrow = 64 if h % 2 == 0 else 32
            rcp = sing.tile([1, S], F32, tag=f"rcp{h}")
            if copy_on_act:
                nc.scalar.activation(rcp, ps[rrow : rrow + 1, :], AF.Identity)
            else:
                nc.vector.tensor_copy(rcp, ps[rrow : rrow + 1, :])
            rr = sing.tile([1, S], F32, tag=f"rr{h}")
            nc.vector.reciprocal_approx_fast(rr, rcp)
            rsc = sing.tile([1, S], F16, tag=f"rsc{h}")
            nc.vector.tensor_scalar_mul(rsc, rr, omg_r[:, h : h + 1])
            if DEBUG:
                nc.sync.dma_start(out=dbg_rraw.ap()[h : h + 1, :], in_=rcp)
                nc.sync.dma_start(out=dbg_rr.ap()[h : h + 1, :], in_=rr)
            return rsc

        def bcast_pair(pp, rsc_a, rsc_b):
            rb = ps_sm.tile([128, S], F32, tag="sm")
            nc.tensor.matmul(rb, onesB, rsc_b, start=True, stop=False)
            nc.tensor.matmul(
                rb[0:64, :],
                ones1h[0:1, 0:64],
                rsc_a,
                start=False,
                stop=True,
                skip_group_check=True,
            )
            rb_sb = sing.tile([128, S], F16, tag=f"rb{pp}")
            if pp == 1:
                nc.scalar.activation(rb_sb, rb, AF.Identity)
            else:
                nc.vector.tensor_copy(rb_sb, rb)
            return rb_sb

        oT = sing.tile([128, MT, S], F16)

        def finish_pair(m, ps_a, ps_b, rb):
            nc.vector.tensor_mul(oT[0:64, m, :], ps_a[0:64, :], rb[0:64, :])
            nc.vector.tensor_mul(oT[64:128, m, :], ps_b[64:128, :], rb[64:128, :])

        # ---- gwv -> bias fold ----
        bo_eff = sing.tile([1, DIM], F16)
        g_sb_dbg, gw_feat_dbg = [], []

        def gwv_path():
            gps = ps_sm.tile([HGH, HGF], F32, tag="sm")
            for jt in range(ST):
                nc.tensor.matmul(
                    gps,
                    wj[:, jt, :],
                    v_aug[:, jt, :, 0:DH],
                    start=(jt == 0),
                    stop=(jt == ST - 1),
                )
            g_sb = sing.tile([HGH, HGF], F16)
            nc.vector.tensor_copy(g_sb, gps)
            g_sb_dbg.append(g_sb)
            gw_feat = sing.tile([128, MT], F16)
            gw_feat_dbg.append(gw_feat)
            for fm in range(MT):
                tp = ps_sm.tile([128, HGH], F16, tag="sm")
                nc.tensor.transpose(
                    tp, g_sb[:, 128 * fm : 128 * (fm + 1)], eye[0:HGH, 0:HGH]
                )
                nc.vector.tensor_copy(
                    gw_feat[0:64, fm : fm + 1], tp[0:64, 2 * fm : 2 * fm + 1]
                )
                nc.vector.tensor_copy(
                    gw_feat[64:128, fm : fm + 1], tp[64:128, 2 * fm + 1 : 2 * fm + 2]
                )
            yg = ps_sm.tile([1, DIM], F32, tag="sm")
            nc.tensor.matmul(yg, gw_feat[:, 0:1], wo[:, 0, :], start=True, stop=False)
            nc.tensor.matmul(yg, gw_feat[:, 1:2], wo[:, 1, :], start=False, stop=False)
            nc.tensor.matmul(yg, ones1h[0:1, 0:1], bo_row, start=False, stop=True)
            nc.vector.tensor_copy(bo_eff, yg)

        # ---- out-projection: bias via K=1 matmul, y DMA straight from
        # PSUM (fp32; host casts/sums) ----
        def outproj(it):
            yps = ps_ea.tile([128, DIM], F32, tag="ea")
            for fm in range(MT):
                nc.tensor.matmul(
                    yps,
                    oT[:, fm, 128 * it : 128 * (it + 1)],
                    wo[:, fm, :],
                    start=(fm == 0),
                    stop=False,
                )
            nc.tensor.matmul(yps, ones1h, bo_eff, start=False, stop=True)
            ysb = ypool.tile([128, DIM], F16, tag="y")
            if it % 2 == 0:
                nc.scalar.activation(ysb, yps, AF.Identity)
            else:
                nc.vector.tensor_copy(ysb, yps)
            nc.sync.dma_start(out=y_d.ap()[128 * it : 128 * (it + 1), :], in_=ysb)

        # ---- main sequence (PE emission order == PE queue order).
        # All scores go as early as possible: the ACT exp stream is the
        # pipeline pacer and must never starve.  eav/gwv/bcast fill the
        # PE while exps run; the pair-1 tail row ops ride on ACT (idle
        # after the last exp).
        with nc.named_scope("proj"):
            proj_one(kT_sb, wk, 0, True)
            pos_l2()
            proj_one(qT_sb, wq, 0, False)
            pos_l3()
        with nc.named_scope("attn"):
            e0 = scores(0)
            proj_one(kT_sb, wk, 1, False)
            proj_one(qT_sb, wq, 1, False)
            e1 = scores(1)
            with nc.named_scope("projv"):
                proj_v()
            e2 = scores(2)
            e3 = scores(3)
            wj_transposes()
            ps_a0 = eav(0, e0)
            rsc0 = rsc_row(0, ps_a0)
            ps_b0 = eav(1, e1)
            rsc1 = rsc_row(1, ps_b0)
            ps_a1 = eav(2, e2)
            rsc2 = rsc_row(2, ps_a1)
            rb0 = bcast_pair(0, rsc0, rsc1)
            finish_pair(0, ps_a0, ps_b0, rb0)
            with nc.named_scope("gwv"):
                gwv_path()
            ps_b1 = eav(3, e3)
            rsc3 = rsc_row(3, ps_b1, copy_on_act=True)
            rb1 = bcast_pair(1, rsc2, rsc3)
            finish_pair(1, ps_a1, ps_b1, rb1)
        with nc.named_scope("outproj"):
            for it in range(ST):
                outproj(it)
    nc.compile()
    return nc


def _get_program():
    if "nc" not in _CACHE:
        _CACHE["nc"] = _build_program()
    return _CACHE["nc"]


def _ktile(a, dtype=np.float16):
    # [K*128, n] -> [128, K*n] (per-partition-contiguous k-tile layout)
    k = a.shape[0] // 128
    return np.ascontiguousarray(
        a.reshape(k, 128, a.shape[1]).transpose(1, 0, 2).reshape(128, -1).astype(dtype)
    )


def _make_in_maps(inputs):
    f = lambda a: np.ascontiguousarray(np.asarray(a), dtype=np.float32)
    x = f(inputs["x"])
    pos = f(inputs["pos"])
    Wq, Wk, Wv, Wo = f(inputs["Wq"]), f(inputs["Wk"]), f(inputs["Wv"]), f(inputs["Wo"])
    bo = f(inputs["bo"])
    Wp1, bp1 = f(inputs["Wp1"]), f(inputs["bp1"])
    Wp2, bp2 = f(inputs["Wp2"]), f(inputs["bp2"])
    Wh, gate = f(inputs["Wh"]), f(inputs["gate"])
    # pad the tiny pos-MLP first layer to 4 outputs (fp32r even-size rule)
    Wp1 = np.concatenate([Wp1, np.zeros((POS_DIM, 1), np.float32)], axis=1)
    bp1 = np.concatenate([bp1, np.zeros(1, np.float32)])
    eye = np.eye(128, dtype=np.float16)

    in_maps = []
    for c in range(NCORES):
        b, hg = c // 2, c % 2
        cs = slice(HGF * hg, HGF * (hg + 1))
        in_maps.append(
            {
                "xT": _ktile(x[b].T),
                "Wq": _ktile(Wq[:, cs]),
                "Wk": _ktile(Wk[:, cs]),
                "Wv": _ktile(Wv[:, cs]),
                "Wo": _ktile(Wo[cs, :]),
                "bo": (bo if hg == 0 else np.zeros_like(bo)).astype(np.float16)[None, :],
                "eye": eye,
                "posT": np.ascontiguousarray(pos[b].T),
                "Wp1": Wp1,
                "bp1": bp1,
                "Wp2": Wp2,
                "bp2": bp2,
                "Wh": np.ascontiguousarray(Wh[:, HGH * hg : HGH * (hg + 1)]),
                "gate_r": np.ascontiguousarray(
                    gate[HGH * hg : HGH * (hg + 1)][None, :]
                ),
                "gate_c": np.ascontiguousarray(gate[HGH * hg : HGH * (hg + 1)]),
            }
        )
    return in_maps


def run(inputs, trace=False):
    """Run on 8 NeuronCores; returns (out [B,S,DIM] fp32, BassKernelResults)."""
    from concourse.bass_utils import run_bass_kernel_spmd

    nc = _get_program()
    in_maps = _make_in_maps(inputs)
    res = run_bass_kernel_spmd(
        nc, in_maps, core_ids=list(range(NCORES)), trace=trace
    )
    out = np.empty((B, S, DIM), np.float32)
    for b in range(B):
        out[b] = res.results[2 * b]["y"].astype(np.float32) + res.results[
            2 * b + 1
        ]["y"].astype(np.float32)
    return out, res


def kernel(**inputs):
    out, _ = run(inputs, trace=False)
    return out
